# revision 18
# baseline (speedup 1.0000x reference)
# Trainium2 Bass kernel for DCNNv2 GNN message passing.
#
# Strategy: shard the G (graph) axis data-parallel across 8 cores; replicate
# the 10000x128 impact table and the small weights.  On each core:
#   Phase A: T2M = impact @ M.T, T2W = impact @ W.T  (bf16, local DRAM,
#            separate tables; all T2M writes are flushed before any T2W
#            write so the neighbour gathers can start earlier).  All phase-A
#            DMAs use HWDGE (sync engine) + DVE casts so the GPSIMD Q7
#            cores never load the strided-dma ucode lib -- the gather lib
#            loads once at program start instead of swapping mid-kernel.
#   Phase B: per chunk of 1024 (g,k) nodes, 3 dma_gather calls (nbr j0-3,
#            nbr j4-7 from T2M; self from T2W), round-robin across the 4
#            SWDGE queues (descriptor generation at ~8ns/idx/queue is the
#            kernel's dominant cost).  PE identity-matmul accumulates the 9
#            rows per node into PSUM, ACT applies relu, PE ones-matmul sums
#            the 64 nodes per graph, softmax -> E rows; each half of E is
#            AllGathered as soon as it completes (rep0 hides under phase B).
#   Phase D: ext-neighbour sums via count-matmuls (lhsT = E tile from SBUF,
#            rhs = host-built multiplicity counts) -- no gather descriptors
#            on the critical tail.  U/V matmuls, relu, softmax -> X;
#            AllGather X (natural row layout).
#   Phase E: pair rows via one-hot selection matmuls from X in SBUF (the
#            transposed output layout feeds the W1 matmuls directly).
# Host side only marshals data (sharding, int16 index packing, count/one-hot
# matrix construction, weight transposes); all FLOPs happen on device.

import numpy as np

D = 128
NT = 10000       # impact rows
G = 2000
K = 64
DIN = 8
DEXT = 16
B = 1024
NCORES = 8
GL = G // NCORES           # 250 graphs per core
NKL = GL * K               # 16000 (g,k) rows per core
CHUNK = 1024               # gk rows per gather chunk
NSTREAM = DIN + 1          # 8 neighbor slots + self
BL = B // NCORES           # 128 batch pairs per core
NQ = 4                     # SWDGE queues
NBLKC = CHUNK // D         # 16 blocks of 128 nodes per full chunk
KSW = 2 * NBLKC            # 32 graph rows per full chunk
GT = 16                    # graph tiles (2048 rows incl pad) for E/X matmuls
GPAD = GT * D              # 2048 rows in E_full/X_full (2000 data + 48 pad)

_PROGRAM_CACHE = {}


def _chunks():
    out = []
    lo = 0
    while lo < NKL:
        hi = min(lo + CHUNK, NKL)
        out.append((lo, hi))
        lo = hi
    return out


def _idx_cols(n):
    return n * NSTREAM // 16   # int16 idx columns for n gk rows


def _wrap16(flat_i16):
    """Pack a flat int16 index stream for dma_gather: element i at
    [i % 16, i // 16], replicated across the 8 groups of 16 partitions."""
    a = np.asarray(flat_i16, dtype=np.int16).reshape(-1, 16).T   # [16, n/16]
    return np.ascontiguousarray(np.tile(a, (8, 1)))              # [128, n/16]


def build_program():
    import concourse.bacc as bacc
    import concourse.tile as tile
    import concourse.mybir as mybir

    f32 = mybir.dt.float32
    bf16 = mybir.dt.bfloat16
    i16 = mybir.dt.int16
    AF = mybir.ActivationFunctionType
    ALU = mybir.AluOpType

    nc = bacc.Bacc(
        "TRN2",
        target_bir_lowering=False,
        debug=False,
        enable_asserts=False,
        num_devices=NCORES,
        num_swdge_queues=NQ,
    )

    # ---- external inputs (per core) ----
    impact_T = nc.dram_tensor("impact_T", [D, NT], f32, kind="ExternalInput").ap()
    rhs_MW = nc.dram_tensor("rhs_MW", [D, 2 * D], f32, kind="ExternalInput").ap()
    UT = nc.dram_tensor("UT", [D, D], f32, kind="ExternalInput").ap()
    VT = nc.dram_tensor("VT", [D, D], f32, kind="ExternalInput").ap()
    W1mT = nc.dram_tensor("W1mT", [D, D], f32, kind="ExternalInput").ap()
    W1sT = nc.dram_tensor("W1sT", [D, D], f32, kind="ExternalInput").ap()
    W2T = nc.dram_tensor("W2T", [D, 2], f32, kind="ExternalInput").ap()
    b1_in = nc.dram_tensor("b1", [D, 1], f32, kind="ExternalInput").ap()
    b2_in = nc.dram_tensor("b2", [2, 1], f32, kind="ExternalInput").ap()
    ident_in = nc.dram_tensor("ident", [D, D], f32, kind="ExternalInput").ap()
    identh_in = nc.dram_tensor("identh", [D, D], bf16, kind="ExternalInput").ap()
    ks_in = nc.dram_tensor("ks", [D, NBLKC * KSW], f32, kind="ExternalInput").ap()
    zeros_in = nc.dram_tensor("zeros", [D, D], bf16, kind="ExternalInput").ap()
    cnt_ext_in = nc.dram_tensor("cnt_ext", [D, GT * GL], bf16,
                                kind="ExternalInput").ap()
    sel_pair_in = nc.dram_tensor("sel_pair", [D, GT * 2 * BL], bf16,
                                 kind="ExternalInput").ap()

    n_big_cols = sum(_idx_cols(hi - lo) for lo, hi in _chunks())
    idx_big_in = nc.dram_tensor("idx_big", [D, n_big_cols], i16,
                                kind="ExternalInput").ap()

    out_dram = nc.dram_tensor("out", [BL, 2], f32, kind="ExternalOutput").ap()

    with tile.TileContext(nc) as tc:
        # ---- long-lived DRAM scratch ----
        NTT = (NT + D - 1) // D           # 79 tile-slots per partition
        T2M_dram, _f0 = tc.tile([D * NTT, D], bf16, space="DRAM",
                                name="T2M_table")
        T2W_dram, _f1 = tc.tile([D * NTT, D], bf16, space="DRAM",
                                name="T2W_table")
        E_loc_dram, _f2 = tc.tile([GL, D], bf16, space="DRAM", name="E_loc")
        E_full, _f3 = tc.tile([GPAD, D], bf16, space="DRAM", name="E_full")
        X_loc_dram, _f4 = tc.tile([GL, D], bf16, space="DRAM", name="X_loc")
        X_full, _f5 = tc.tile([GPAD, D], bf16, space="DRAM", name="X_full")

        # ---- long-lived SBUF constants ----
        cpool_cm = tc.tile_pool(name="consts", bufs=1)
        cpool = cpool_cm.__enter__()
        # tiny dummy gather fires as soon as its idx columns land: forces
        # the Q7 gather ucode lib to load during phase A instead of after it
        # (the lib load otherwise queues behind the first real gather's
        # T2M-ready semaphore wait).
        idx_big_sb = cpool.tile([D, n_big_cols], i16, name="idx_big_sb")
        nc.sync.dma_start(out=idx_big_sb[:, 0:8],
                          in_=idx_big_in[:, 0:8])
        qctr = [0]

        def _next_q():
            q = qctr[0] % NQ
            qctr[0] += 1
            return q

        dummy_gt = cpool.tile([D, 1, D], bf16, name="dummy_gt")
        nc.gpsimd.dma_gather(
            out_ap=dummy_gt[:], in_ap=T2M_dram[:],
            idxs_ap=idx_big_sb[:, 0:8],
            num_idxs=128, num_idxs_reg=128, elem_size=D, queue_num=_next_q(),
        )
        nc.sync.dma_start(out=idx_big_sb[:, 8:n_big_cols],
                          in_=idx_big_in[:, 8:n_big_cols])
        ident_sb = cpool.tile([D, D], f32, name="ident_sb")
        nc.sync.dma_start(out=ident_sb[:], in_=ident_in[:])
        identh_sb = cpool.tile([D, D], bf16, name="identh_sb")
        nc.sync.dma_start(out=identh_sb[:], in_=identh_in[:])
        ks_f = cpool.tile([D, NBLKC * KSW], f32, name="ks_f")
        nc.sync.dma_start(out=ks_f[:], in_=ks_in[:])
        ks_sb = cpool.tile([D, NBLKC * KSW], bf16, name="ks_sb")
        nc.vector.tensor_copy(out=ks_sb[:], in_=ks_f[:])

        # zero the pad tails (rows 2000..2047) of E_full/X_full so the
        # padded [0:2048] reloads never pull NaN garbage into the 0-count
        # matmul lanes.
        zeros_sb = cpool.tile([D, D], bf16, name="zeros_sb")
        nc.sync.dma_start(out=zeros_sb[:], in_=zeros_in[:])
        nc.sync.dma_start(out=E_full[G:GPAD, :], in_=zeros_sb[0:GPAD - G, :])
        nc.sync.dma_start(out=X_full[G:GPAD, :], in_=zeros_sb[0:GPAD - G, :])

        # =========================== Phase A ===========================
        # T2M[t] = impact[t] @ M.T ; T2W[t] = impact[t] @ W.T.
        # HWDGE f32 loads + DVE casts; stage tiles retained so all T2M
        # writes flush before the first T2W write.
        n_tiles = (NT + D - 1) // D        # 79
        GRP = 16
        groups = []
        t = 0
        while t < n_tiles:
            ns = min(GRP, n_tiles - t)
            if 8 < n_tiles - t <= GRP:
                ns = 8    # split the final group: shorter copy/write tail
            groups.append((t, ns))
            t += ns

        with tc.tile_pool(name="phaseA_stage", bufs=1) as spool, \
             tc.tile_pool(name="phaseA_sb", bufs=2) as apool, \
             tc.tile_pool(name="phaseA_ps", bufs=4, space="PSUM") as appool:
            mw_f = apool.tile([D, 2 * D], f32, name="mw_f")
            nc.scalar.dma_start(out=mw_f[:], in_=rhs_MW[:])
            mw_sb = apool.tile([D, 2 * D], bf16, name="mw_sb")
            nc.vector.tensor_copy(out=mw_sb[:], in_=mw_f[:])

            stage_tiles = []
            for gi, (t0, ns) in enumerate(groups):
                gw = min(ns * D, NT - t0 * D)
                imp_f = apool.tile([D, GRP * D], f32, tag="impf")
                nc.scalar.dma_start(out=imp_f[:, :gw],
                                    in_=impact_T[:, t0 * D:t0 * D + gw])
                imp_g = apool.tile([D, GRP * D], bf16, tag="impg")
                nc.vector.tensor_copy(out=imp_g[:, :gw], in_=imp_f[:, :gw])
                stage = spool.tile([D, ns, 2 * D], bf16, name=f"stage{gi}")
                for s in range(ns):
                    tw = min(D, NT - (t0 + s) * D)      # 128, last tile 16
                    psA = appool.tile([D, 2 * D], f32, tag="psA")
                    nc.tensor.matmul(
                        out=psA[:tw, :],
                        lhsT=imp_g[:, s * D:s * D + tw],
                        rhs=mw_sb[:],
                        start=True, stop=True,
                    )
                    if s % 2 == 0:
                        nc.scalar.copy(out=stage[:tw, s, :], in_=psA[:tw, :])
                    else:
                        nc.vector.tensor_copy(out=stage[:tw, s, :],
                                              in_=psA[:tw, :])
                stage_tiles.append((t0, ns, stage))

            # flush ALL T2M writes first, then the T2W writes.  The table
            # is partition-major (row' = p*NTT + t), so each group write is
            # one contiguous ns*256B run per partition (128 descriptors).
            for half, table in ((0, T2M_dram), (1, T2W_dram)):
                for t0, ns, stage in stage_tiles:
                    nf = ns if (t0 + ns) * D <= NT else ns - 1
                    if nf > 0:
                        nc.sync.dma_start(
                            out=table[:, :].rearrange("(p t) d -> p t d", p=D)
                                [:, t0:t0 + nf, :],
                            in_=stage[:, :nf, half * D:(half + 1) * D],
                        )
                    if nf < ns:  # partial last tile (16 valid rows)
                        pw = NT - (t0 + nf) * D
                        nc.sync.dma_start(
                            out=table[:, :].rearrange("(p t) d -> p t d", p=D)
                                [0:pw, t0 + nf:t0 + nf + 1, :],
                            in_=stage[:pw, nf:nf + 1,
                                      half * D:(half + 1) * D],
                        )

        # preload the phase-D/E count and selection matrices on the vector
        # engine's HWDGE queue so the transfers hide under phase B.
        cnt_ext_sb = cpool.tile([D, GT * GL], bf16, name="cnt_ext_sb")
        nc.sync.dma_start(out=cnt_ext_sb[:], in_=cnt_ext_in[:])
        sel_sb = cpool.tile([D, GT * 2 * BL], bf16, name="sel_sb")
        nc.sync.dma_start(out=sel_sb[:], in_=sel_pair_in[:])

        # =========================== Phase B ===========================
        # per-graph pre-softmax rows accumulate in E_pre ([128, 2, D]:
        # graph r on partition r % 128, rep r // 128); per-rep softmax +
        # AllGather as soon as each half completes.
        E_pre = cpool.tile([D, 2, D], f32, name="E_pre")
        E_out = cpool.tile([D, 2, D], bf16, name="E_out")
        chunks = _chunks()
        chunk_col0 = []
        _c = 0
        for lo, hi in chunks:
            chunk_col0.append(_c)
            _c += _idx_cols(hi - lo)

        with tc.tile_pool(name="gpool", bufs=4) as gpool, \
             tc.tile_pool(name="bpool", bufs=4) as bpool, \
             tc.tile_pool(name="bpsum", bufs=4, space="PSUM") as bppool, \
             tc.tile_pool(name="b2psum", bufs=2, space="PSUM") as b2ppool:
            for ci, (lo, hi) in enumerate(chunks):
                nb = hi - lo
                nblk = nb // D                     # 8 (last chunk 5)
                jcols = nb // 16
                col0 = chunk_col0[ci]
                gt = gpool.tile([D, NSTREAM * nblk, D], bf16, tag="gt",
                                name=f"gt{ci}")
                # 9 gather calls of <=1024 idxs each (the SWDGE descriptor
                # ring holds only 1024 descriptors per queue): neighbour
                # slots j0-7 from T2M, self rows from T2W.
                for j in range(NSTREAM):
                    nc.gpsimd.dma_gather(
                        out_ap=gt[:, j * nblk:(j + 1) * nblk, :],
                        in_ap=T2W_dram[:] if j == DIN else T2M_dram[:],
                        idxs_ap=idx_big_sb[:, col0 + j * jcols:
                                           col0 + (j + 1) * jcols],
                        num_idxs=nb, num_idxs_reg=nb, elem_size=D,
                        queue_num=_next_q(),
                    )

                # all accumulate matmuls first (identity stays stationary),
                # then relu, then the k-sum matmuls.
                pss = []
                for h in range(0, nblk, 4):
                    hw = min(4, nblk - h)
                    ps = bppool.tile([D, 4 * D], f32, tag="psB")
                    for j in range(NSTREAM):
                        nc.tensor.matmul(
                            out=ps[:, :hw * D],
                            lhsT=identh_sb[:],
                            rhs=gt[:, j * nblk + h: j * nblk + h + hw, :],
                            start=(j == 0), stop=(j == NSTREAM - 1),
                        )
                    pss.append((h, hw, ps))
                msgs = []
                for h, hw, ps in pss:
                    msg = bpool.tile([D, 4 * D], bf16, tag="msg")
                    nc.scalar.activation(out=msg[:, :hw * D], in_=ps[:, :hw * D],
                                         func=AF.Relu)
                    msgs.append((h, hw, msg))
                ps2 = b2ppool.tile([KSW, D], f32, tag="ps2")
                for h, hw, msg in msgs:
                    # k-sum: 64 nodes per graph -> 2 graph rows per block
                    for bi in range(hw):
                        b = h + bi
                        nc.tensor.matmul(
                            out=ps2[:],
                            lhsT=ks_sb[:, b * KSW:(b + 1) * KSW],
                            rhs=msg[:, bi * D:(bi + 1) * D],
                            start=(b == 0), stop=(b == nblk - 1),
                        )
                ng = nb // K                       # 16 (last chunk 10)
                r0 = ci * KSW                      # first graph row of chunk
                s2 = bpool.tile([KSW, D], f32, tag="s2")
                nc.vector.tensor_copy(out=s2[:ng, :], in_=ps2[:ng, :])
                nc.sync.dma_start(
                    out=E_pre[r0 % D:r0 % D + ng, r0 // D, :],
                    in_=s2[:ng, :])

                # as each half of E_pre completes, softmax it and kick off
                # its AllGather so the first one overlaps the second half of
                # phase B.  E_full layout: graph (c, r) -> c*128 + r for
                # r < 128, else 1024 + c*122 + (r - 128); see _prep_in_maps.
                if ci in (D // KSW - 1, len(chunks) - 1):
                    rep = 0 if ci == D // KSW - 1 else 1
                    gw = D if rep == 0 else GL - D
                    nmx = bpool.tile([D, 1], f32, tag="nmxE")
                    nc.vector.tensor_reduce(out=nmx[:gw, :],
                                            in_=E_pre[:gw, rep, :],
                                            axis=mybir.AxisListType.X,
                                            op=ALU.max, negate=True)
                    sm = bpool.tile([D, 1], f32, tag="smE")
                    ex = bpool.tile([D, D], f32, tag="exE")
                    nc.scalar.activation(out=ex[:gw, :], in_=E_pre[:gw, rep, :],
                                         func=AF.Exp, bias=nmx[:gw, :],
                                         accum_out=sm[:gw, :])
                    rs = bpool.tile([D, 1], f32, tag="rsE")
                    nc.vector.reciprocal(out=rs[:gw, :], in_=sm[:gw, :])
                    nc.vector.tensor_scalar_mul(out=E_out[:gw, rep, :],
                                                in0=ex[:gw, :],
                                                scalar1=rs[:gw, :])
                    nc.sync.dma_start(out=E_loc_dram[rep * D:rep * D + gw, :],
                                      in_=E_out[:gw, rep, :])
                    nc.gpsimd.collective_compute(
                        "AllGather", ALU.bypass,
                        replica_groups=[list(range(NCORES))],
                        ins=[E_loc_dram[rep * D:rep * D + gw, :].opt()],
                        outs=[E_full[rep * NCORES * D:
                                     rep * NCORES * D + NCORES * gw, :].opt()],
                    )

        # =========================== Phase D ===========================
        with tc.tile_pool(name="dpool", bufs=1) as dpool, \
             tc.tile_pool(name="dpsum", bufs=1, space="PSUM") as dppool:
            UT_sb = dpool.tile([D, D], f32, name="UT_sb")
            nc.sync.dma_start(out=UT_sb[:], in_=UT[:])
            VT_sb = dpool.tile([D, D], f32, name="VT_sb")
            nc.sync.dma_start(out=VT_sb[:], in_=VT[:])

            # U-part primes early: transpose local E (in SBUF since phase B)
            # and start the U matmul; it only depends on E_out, not on the
            # AllGathers.  ET is flat [D, 256] so one matmul covers all 250
            # graph columns (partial-range psum starts are illegal).
            ET = dpool.tile([D, 2 * D], f32, name="ET")
            ps_x = dppool.tile([D, 2 * D], f32, name="ps_x")
            for rep in range(2):
                pt = dppool.tile([D, D], bf16, name=f"ptDh{rep}")
                nc.tensor.transpose(out=pt[:], in_=E_out[:, rep, :],
                                    identity=identh_sb[:])
                nc.vector.tensor_copy(out=ET[:, rep * D:(rep + 1) * D], in_=pt[:])
            nc.tensor.matmul(out=ps_x[:, 0:GL], lhsT=UT_sb[:], rhs=ET[:, 0:GL],
                             start=True, stop=False)

            # ext-neighbour sums: count-matmuls over the 16 E tiles.
            # E_sb[p, t, :] = E_full[t*128 + p, :]; pad rows are zero.
            # Split load: tiles 0-7 (rep0 rows) only depend on the first
            # AllGather, so their load and matmuls hide under the second.
            E_sb = dpool.tile([D, GT, D], bf16, name="E_sb")
            nc.sync.dma_start(
                out=E_sb[:, 0:GT // 2, :],
                in_=E_full[0:GPAD // 2, :].rearrange("(t p) d -> p t d", p=D))
            nc.sync.dma_start(
                out=E_sb[:, GT // 2:GT, :],
                in_=E_full[GPAD // 2:GPAD, :].rearrange("(t p) d -> p t d", p=D))
            ps_nb = dppool.tile([D, GL], f32, name="ps_nb")
            for t2 in range(GT):
                nc.tensor.matmul(
                    out=ps_nb[:],
                    lhsT=E_sb[:, t2, :],
                    rhs=cnt_ext_sb[:, t2 * GL:(t2 + 1) * GL],
                    start=(t2 == 0), stop=(t2 == GT - 1),
                )
            nbr_sb = dpool.tile([D, GL], f32, name="nbr_sb")
            nc.scalar.copy(out=nbr_sb[:], in_=ps_nb[:])

            # V-part accumulates onto the U-part psum; relu.
            nc.tensor.matmul(out=ps_x[:, 0:GL], lhsT=VT_sb[:], rhs=nbr_sb[:],
                             start=False, stop=True)
            extT = dpool.tile([D, 2 * D], f32, name="extT")
            nc.scalar.activation(out=extT[:, 0:GL], in_=ps_x[:, 0:GL],
                                 func=AF.Relu)

            # transpose back -> [g, d], softmax rows -> X (bf16)
            Xg = dpool.tile([D, 2, D], bf16, name="Xg")
            xe = dpool.tile([D, D], f32, name="xe")
            nmx2 = dpool.tile([D, 1], f32, name="nmx2")
            sm2 = dpool.tile([D, 1], f32, name="sm2")
            rs2 = dpool.tile([D, 1], f32, name="rs2")
            for rep in range(2):
                gw = D if rep == 0 else GL - D
                pt3 = dppool.tile([D, D], f32, name=f"ptD{rep}")
                nc.tensor.transpose(out=pt3[:], in_=extT[:, rep * D:(rep + 1) * D],
                                    identity=ident_sb[:])
                nc.vector.tensor_reduce(out=nmx2[:gw, :], in_=pt3[:gw, :],
                                        axis=mybir.AxisListType.X,
                                        op=ALU.max, negate=True)
                nc.scalar.activation(out=xe[:gw, :], in_=pt3[:gw, :],
                                     func=AF.Exp, bias=nmx2[:gw, :],
                                     accum_out=sm2[:gw, :])
                nc.vector.reciprocal(out=rs2[:gw, :], in_=sm2[:gw, :])
                nc.vector.tensor_scalar_mul(out=Xg[:gw, rep, :],
                                            in0=xe[:gw, :],
                                            scalar1=rs2[:gw, :])
            nc.sync.dma_start(out=X_loc_dram[0:D, :], in_=Xg[:, 0, :])
            nc.sync.dma_start(out=X_loc_dram[D:GL, :], in_=Xg[:GL - D, 1, :])

        # ---- AllGather X shards (natural global row layout) ----
        nc.gpsimd.collective_compute(
            "AllGather", ALU.bypass,
            replica_groups=[list(range(NCORES))],
            ins=[X_loc_dram[:].opt()],
            outs=[X_full[0:G, :].opt()],
        )

        # =========================== Phase E ===========================
        with tc.tile_pool(name="epool", bufs=1) as epool, \
             tc.tile_pool(name="epsum", bufs=1, space="PSUM") as eppool:
            X_sb = epool.tile([D, GT, D], bf16, name="X_sb")
            nc.sync.dma_start(
                out=X_sb[:],
                in_=X_full[0:GPAD, :].rearrange("(t p) d -> p t d", p=D))

            # pair rows via one-hot selection matmuls: out[d, pair-col];
            # cols 0:128 = X[batch[:,0]].T, cols 128:256 = X[batch[:,1]].T
            ps_p = eppool.tile([D, 2 * BL], f32, name="ps_p")
            for t2 in range(GT):
                nc.tensor.matmul(
                    out=ps_p[:],
                    lhsT=X_sb[:, t2, :],
                    rhs=sel_sb[:, t2 * 2 * BL:(t2 + 1) * 2 * BL],
                    start=(t2 == 0), stop=(t2 == GT - 1),
                )
            e12 = epool.tile([D, 2 * BL], f32, name="e12")
            nc.scalar.copy(out=e12[:], in_=ps_p[:])
            mT = epool.tile([D, BL], f32, name="mT")
            nc.vector.tensor_mul(out=mT[:], in0=e12[:, 0:BL],
                                 in1=e12[:, BL:2 * BL])
            sT = epool.tile([D, BL], f32, name="sT")
            nc.vector.tensor_add(out=sT[:], in0=e12[:, 0:BL],
                                 in1=e12[:, BL:2 * BL])

            W1mT_sb = epool.tile([D, D], f32, name="W1mT_sb")
            nc.sync.dma_start(out=W1mT_sb[:], in_=W1mT[:])
            W1sT_sb = epool.tile([D, D], f32, name="W1sT_sb")
            nc.sync.dma_start(out=W1sT_sb[:], in_=W1sT[:])
            W2T_sb = epool.tile([D, 2], f32, name="W2T_sb")
            nc.sync.dma_start(out=W2T_sb[:], in_=W2T[:])
            b1_sb = epool.tile([D, 1], f32, name="b1_sb")
            nc.sync.dma_start(out=b1_sb[:], in_=b1_in[:])
            b2_sb = epool.tile([2, 1], f32, name="b2_sb")
            nc.sync.dma_start(out=b2_sb[:], in_=b2_in[:])

            ps4 = eppool.tile([D, D], f32, name="ps4")
            nc.tensor.matmul(out=ps4[:], lhsT=W1mT_sb[:], rhs=mT[:],
                             start=True, stop=False)
            nc.tensor.matmul(out=ps4[:], lhsT=W1sT_sb[:], rhs=sT[:],
                             start=False, stop=True)
            hT = epool.tile([D, D], f32, name="hT")
            nc.scalar.activation(out=hT[:], in_=ps4[:], func=AF.Relu,
                                 bias=b1_sb[:])

            ps5 = eppool.tile([2, D], f32, name="ps5")
            nc.tensor.matmul(out=ps5[:], lhsT=W2T_sb[:], rhs=hT[:],
                             start=True, stop=True)
            lgT = epool.tile([2, D], f32, name="lgT")
            nc.vector.tensor_scalar_add(out=lgT[:], in0=ps5[:], scalar1=b2_sb[:])

            ps6 = eppool.tile([D, 2], f32, name="ps6")
            nc.tensor.transpose(out=ps6[:], in_=lgT[:], identity=ident_sb[:2, :2])
            lg = epool.tile([D, 2], f32, name="lg")
            nc.vector.tensor_copy(out=lg[:], in_=ps6[:])

            nmx3 = epool.tile([D, 1], f32, name="nmx3")
            nc.vector.tensor_reduce(out=nmx3[:], in_=lg[:],
                                    axis=mybir.AxisListType.X,
                                    op=ALU.max, negate=True)
            ex3 = epool.tile([D, 2], f32, name="ex3")
            sm3 = epool.tile([D, 1], f32, name="sm3")
            nc.scalar.activation(out=ex3[:], in_=lg[:], func=AF.Exp,
                                 bias=nmx3[:], accum_out=sm3[:])
            rs3 = epool.tile([D, 1], f32, name="rs3")
            nc.vector.reciprocal(out=rs3[:], in_=sm3[:])
            nc.vector.tensor_scalar_mul(out=ex3[:], in0=ex3[:], scalar1=rs3[:])
            nc.sync.dma_start(out=out_dram[:], in_=ex3[:])

        cpool_cm.__exit__(None, None, None)
        for f in (_f0, _f1, _f2, _f3, _f4, _f5):
            f()

    nc.compile()
    return nc


def _gmap():
    """global graph id -> E_full row (half-segmented rank-major layout
    produced by the two partial AllGathers in phase B)."""
    gc, gr = np.divmod(np.arange(G, dtype=np.int64), GL)
    seg = np.minimum(gr // D, 1)
    seg_base = np.array([0, NCORES * D], np.int64)
    seg_rows = np.array([D, GL - D], np.int64)
    return seg_base[seg] + gc * seg_rows[seg] + (gr - seg * D)


def _prep_in_maps(inputs):
    batch = np.asarray(inputs["batch"])
    node_type = np.asarray(inputs["node_type"])
    nbr_type = np.asarray(inputs["nbr_type"])
    ext_nbr = np.asarray(inputs["ext_nbr"])
    impact = np.asarray(inputs["impact"], dtype=np.float32)
    W = np.asarray(inputs["W"], dtype=np.float32)
    M = np.asarray(inputs["M"], dtype=np.float32)
    U = np.asarray(inputs["U"], dtype=np.float32)
    V = np.asarray(inputs["V"], dtype=np.float32)
    W1 = np.asarray(inputs["W1"], dtype=np.float32)
    b1 = np.asarray(inputs["b1"], dtype=np.float32)
    W2 = np.asarray(inputs["W2"], dtype=np.float32)
    b2 = np.asarray(inputs["b2"], dtype=np.float32)

    ks = np.zeros((D, NBLKC * KSW), dtype=np.float32)
    for bi in range(NBLKC):
        ks[:K, bi * KSW + 2 * bi] = 1.0
        ks[K:, bi * KSW + 2 * bi + 1] = 1.0

    import ml_dtypes
    ident = np.eye(D, dtype=np.float32)
    identh = ident.astype(ml_dtypes.bfloat16)
    shared = dict(
        impact_T=np.ascontiguousarray(impact.T),
        rhs_MW=np.ascontiguousarray(np.concatenate([M.T, W.T], axis=1)),
        UT=np.ascontiguousarray(U.T),
        VT=np.ascontiguousarray(V.T),
        W1mT=np.ascontiguousarray(W1[:, :D].T),
        W1sT=np.ascontiguousarray(W1[:, D:].T),
        W2T=np.ascontiguousarray(W2.T),
        b1=np.ascontiguousarray(b1.reshape(D, 1)),
        b2=np.ascontiguousarray(b2.reshape(2, 1)),
        ident=ident,
        identh=np.ascontiguousarray(identh),
        ks=ks,
        zeros=np.zeros((D, D), dtype=ml_dtypes.bfloat16),
    )

    gmap = _gmap()
    # T2 tables are partition-major: impact row v lives at table row
    # (v % 128) * NTT + (v // 128) so phase-A writes are contiguous runs.
    NTT = (NT + D - 1) // D
    in_maps = []
    for c in range(NCORES):
        g0 = c * GL
        nbr = nbr_type[g0:g0 + GL].reshape(NKL, DIN).astype(np.int64)
        slf = node_type[g0:g0 + GL].reshape(NKL).astype(np.int64)
        nbr = (nbr % D) * NTT + nbr // D
        slf = (slf % D) * NTT + slf // D
        parts = []
        for lo, hi in _chunks():
            blocks = [nbr[lo:hi, j] for j in range(DIN)]
            blocks.append(slf[lo:hi])          # self rows: separate T2W table
            parts.append(np.concatenate(blocks))
        idx_big = _wrap16(np.concatenate(parts))

        # ext-neighbour count matrix: cnt_ext[r, t*GL + j] = multiplicity of
        # E_full row (t*128 + r) among the mapped ext neighbours of local
        # graph j.
        rows = gmap[ext_nbr[g0:g0 + GL]]               # [GL, DEXT]
        cnt = np.zeros((GT * D, GL), dtype=np.float32)
        for j in range(GL):
            np.add.at(cnt[:, j], rows[j], 1.0)
        cnt_ext = np.ascontiguousarray(
            cnt.reshape(GT, D, GL).transpose(1, 0, 2).reshape(D, GT * GL)
        ).astype(ml_dtypes.bfloat16)

        # pair one-hot selection: natural X_full layout (row = graph id).
        sel = np.zeros((GT * D, 2 * BL), dtype=np.float32)
        pb = batch[c * BL:(c + 1) * BL]
        sel[pb[:, 0], np.arange(BL)] = 1.0
        sel[pb[:, 1], BL + np.arange(BL)] = 1.0
        sel_pair = np.ascontiguousarray(
            sel.reshape(GT, D, 2 * BL).transpose(1, 0, 2).reshape(D, GT * 2 * BL)
        ).astype(ml_dtypes.bfloat16)

        m = dict(shared)
        m["idx_big"] = idx_big
        m["cnt_ext"] = cnt_ext
        m["sel_pair"] = sel_pair
        in_maps.append(m)
    return in_maps


def kernel(**inputs):
    in_maps = _prep_in_maps(inputs)
    if "nc" not in _PROGRAM_CACHE:
        _PROGRAM_CACHE["nc"] = build_program()
    nc = _PROGRAM_CACHE["nc"]

    from concourse import bass_utils
    res = bass_utils.run_bass_kernel_spmd(nc, in_maps, core_ids=list(range(NCORES)))
    out = np.concatenate([r["out"] for r in res.results], axis=0)
    return out.astype(np.float32)
